# revision 1
# baseline (speedup 1.0000x reference)
"""Causal self-attention Trainium2 kernel (B=4, S=2048, D=1024, H=16).

Sharding: 8 cores = 4 batches x 2 head-groups (8 heads each).
Megatron-style: column-parallel QKV, row-parallel output projection;
the 2-way partial-sum reduce + bias happens on host at gather time.

Device-side schedule (per core, batch b, head-group g), engineered for the
TRN2 timeline model (PE cost = out-free-size, ACT cost = free-size, per-
instruction access-latency overheads, exclusive DMA device):

  - fp16 everywhere on chip (same PE rate as bf16, better accuracy).
  - Startup streams chunked DMAs (wqkv_c, xT_c lo-half) while the first QK
    d-block + V chunks 0-3 accumulate c-outer across 6 PSUM banks, chasing
    the DMA arrivals.
  - Both 64-row head-halves of a chunk share one 2-bank PSUM tile so a
    single Exp activation covers them (halves ACT instruction count).
  - Causal masking: diagonal chunks are processed FIRST within each q-tile;
    after the exp, a DVE multiply by an upper-triangular ones tile zeroes
    the k>q entries, its latency hidden behind the remaining plain chunks.
    One PSUM accumulation group per bank (zero-region rule).
  - V-projection, later QK d-blocks and the output projection are emitted as
    "fillers" between attention chunks so PE never stalls on ACT; PV of
    chunk c-1 is emitted after the scores of chunk c for extra slack.
  - PV lhsT carries [V | ones]: PSUM rows 64:128 accumulate the softmax
    denominator; DVE reciprocal+normalize on eviction.
  - y evicted PSUM->SBUF(fp16) on ACT/DVE (alternating), DMA'd as fp16
    partials; host adds the core-pair + bias in fp32.
"""
import numpy as np
from contextlib import ExitStack

import concourse.bass as bass
import concourse.tile as tile
import concourse.mybir as mybir
from concourse.bass_utils import run_bass_kernel_spmd

B, S, D, H = 4, 2048, 1024, 16
HD = 64          # head dim
HPC = 8          # heads per core
DG = HPC * HD    # 512 dims per head-group
P = 128
NQ = 512         # q-tile width
NCH = S // P     # 16 k-chunks
NJ = S // NQ     # 4 q-tiles
NHI = HPC // 2   # 4 head-pairs per core
DCH = D // P     # 8 contraction chunks
DT = mybir.dt.float16
NPDT = np.float16

_CACHE = {}


def split_waits(nc, maxw=1):
    """walrus here accepts at most 1 sync-wait per instruction; split extras onto NOPs."""
    for fn in nc.m.functions:
        for bb in fn.blocks:
            insts = list(bb.instructions)
            new_list = []
            changed = False
            for inst in insts:
                si = inst.sync_info
                waits = list(si.on_wait) if si and si.on_wait else []
                if len(waits) > maxw:
                    changed = True
                    head, keep = waits[:-maxw], waits[-maxw:]
                    for i in range(0, len(head), maxw):
                        nop = mybir.InstNoOp(
                            name=f"{inst.name}_wsplit{i}",
                            sync_info=mybir.SyncInfo(on_wait=head[i:i + maxw], on_update=[]),
                            bass_nofuse=True, engine=inst.engine)
                        nc.register_instruction(nop)
                        new_list.append(nop)
                    inst.sync_info = mybir.SyncInfo(
                        on_wait=keep,
                        on_update=list(si.on_update) if si.on_update else [])
                new_list.append(inst)
            if changed:
                bb.instructions = new_list


def build():
    nc = bass.Bass(trn_type="TRN2", target_bir_lowering=False, debug=False)
    xT = nc.dram_tensor("xT", [D, S], DT, kind="ExternalInput").ap()
    wqkv = nc.dram_tensor("wqkv", [D, 3 * DG], DT, kind="ExternalInput").ap()
    wo = nc.dram_tensor("wo", [DG, D], DT, kind="ExternalInput").ap()
    tri = nc.dram_tensor("tri", [P, 2, P], DT, kind="ExternalInput").ap()
    y = nc.dram_tensor("y", [S, D], DT, kind="ExternalOutput").ap()

    with tile.TileContext(nc) as tc, ExitStack() as ctx:
        sb = ctx.enter_context(tc.tile_pool(name="sb", bufs=1))
        # PSUM: pp 2x1 bank + pss 2x2 banks + pos 1x2 banks = 8 banks
        pp = ctx.enter_context(tc.tile_pool(name="pp", bufs=2, space="PSUM"))
        pss = ctx.enter_context(tc.tile_pool(name="pss", bufs=2, space="PSUM"))
        pos = ctx.enter_context(tc.tile_pool(name="pos", bufs=1, space="PSUM"))
        pt = ctx.enter_context(tc.tile_pool(name="pt", bufs=6))
        rc = ctx.enter_context(tc.tile_pool(name="rc", bufs=2))
        ys = ctx.enter_context(tc.tile_pool(name="ys", bufs=4))

        # ---- resident SBUF tiles ----
        xT_sb = sb.tile([P, DCH, S], DT)
        wqkv_sb = sb.tile([P, DCH, 3 * DG], DT)
        wq_sb = wqkv_sb[:, :, 0:DG]
        wk_sb = wqkv_sb[:, :, DG:2 * DG]
        wv_sb = wqkv_sb[:, :, 2 * DG:3 * DG]
        wo_sb = sb.tile([P, DG // P, D], DT)
        tri_sb = sb.tile([P, 2, P], DT)
        qT_sb = sb.tile([P, NHI, S], DT)   # [2-head dims, pair, s]
        kT_sb = sb.tile([P, NHI, S], DT)
        v_sb = sb.tile([P, NCH, HPC, P], DT)  # [k part, chunk, head, V|ones]
        oT_sb = sb.tile([P, NHI, S], DT)

        # ---- input DMA stream (ordered for earliest compute start) ----
        # The startup (QK-jj0 + V0-3) touches only xT columns 0:512, so the
        # critical stream is (wqkv_c, xT-q0_c); the remaining column quarters
        # stream behind it, consumed by the (0,*) fillers as they land.
        xTr = xT.rearrange("(c p) s -> c p s", p=P)
        wqkvr = wqkv.rearrange("(c p) d -> c p d", p=P)
        HS = S // 2
        for c in range(DCH):
            nc.sync.dma_start(wqkv_sb[:, c], wqkvr[c])
            nc.sync.dma_start(xT_sb[:, c, 0:NQ], xTr[c][:, 0:NQ])
        for c in range(DCH):
            nc.sync.dma_start(xT_sb[:, c, NQ:HS], xTr[c][:, NQ:HS])
        nc.sync.dma_start(tri_sb[:], tri[:])
        for c in range(DCH):
            nc.sync.dma_start(xT_sb[:, c, HS:S], xTr[c][:, HS:S])
        nc.sync.dma_start(wo_sb[:], wo.rearrange("(c p) o -> p c o", p=P))

        # warm-up: a dozen tiny matmuls anchor the PE busy-ramp origin early
        # so the real startup matmuls dispatch at full p-state
        warm = sb.tile([P, HD], DT)
        nc.gpsimd.memset(warm[:], 0.0)
        wps = pp.tile([P, NQ], mybir.dt.float32, tag="pp", name="wps")
        for _ in range(12):
            nc.tensor.matmul(wps[0:HD, 0:HD], warm[:, 0:HD], warm[:],
                             start=True, stop=True)

        nc.gpsimd.memset(v_sb[:, :, :, HD:], 1.0)

        # ---- startup: QK d-block 0 jj0 + V chunks 0..3, c-outer over 6 PSUM
        # banks, chasing the chunked DMA stream ----
        t0 = pp.tile([P, NQ], mybir.dt.float32, tag="pp", name="su_a")
        t1 = pp.tile([P, NQ], mybir.dt.float32, tag="pp", name="su_b")
        t2 = pss.tile([P, 2, NQ], mybir.dt.float32, tag="ps", name="su_c")
        t3 = pss.tile([P, 2, NQ], mybir.dt.float32, tag="ps", name="su_d")
        qacc = [t0, t1]
        vacc = [t2[:, 0], t2[:, 1], t3[:, 0], t3[:, 1]]
        # V c-steps lag QK by one chunk: keeps the first DMA-gated windows
        # light so PE re-gaps (p-state stays at full rate, see notes)
        for c in range(DCH + 1):
            if c < DCH:
                for t, w_sb in enumerate((wq_sb, wk_sb)):
                    nc.tensor.matmul(
                        qacc[t][:], w_sb[:, c, 0:P], xT_sb[:, c, bass.ts(0, NQ)],
                        start=(c == 0), stop=(c == DCH - 1))
            if c > 0:
                for m in range(4):
                    nc.tensor.matmul(
                        vacc[m][:], xT_sb[:, c - 1, bass.ts(m, P)],
                        wv_sb[:, c - 1, :],
                        start=(c == 1), stop=(c == DCH))
        for t, dst in enumerate((qT_sb, kT_sb)):
            nc.vector.tensor_copy(dst[:, 0, bass.ts(0, NQ)], qacc[t][:])
        for m in range(4):
            nc.vector.tensor_copy(
                v_sb[:, m, :, 0:HD],
                vacc[m][:].rearrange("p (h d) -> p h d", d=HD))

        # ---- filler emitters ----
        def emit_qk_tile(i, jj, qk):
            """One [128,512] QK projection tile: d-block i, s-range jj."""
            w_sb, dst = (wq_sb, qT_sb) if qk == 0 else (wk_sb, kT_sb)
            ps = pp.tile([P, NQ], mybir.dt.float32, tag="pp", name="qk")
            for c in range(DCH):
                nc.tensor.matmul(
                    ps[:], w_sb[:, c, bass.ts(i, P)], xT_sb[:, c, bass.ts(jj, NQ)],
                    start=(c == 0), stop=(c == DCH - 1))
            nc.vector.tensor_copy(dst[:, i, bass.ts(jj, NQ)], ps[:])

        def emit_v(m):
            ps = pp.tile([P, DG], mybir.dt.float32, tag="pp", name="vp")
            for c in range(DCH):
                nc.tensor.matmul(
                    ps[:], xT_sb[:, c, bass.ts(m, P)], wv_sb[:, c, :],
                    start=(c == 0), stop=(c == DCH - 1))
            nc.vector.tensor_copy(
                v_sb[:, m, :, 0:HD], ps[:].rearrange("p (h d) -> p h d", d=HD))

        def emit_outproj_unit(m, n, act=False):
            ps = pp.tile([P, NQ], mybir.dt.float32, tag="pp", name="yp")
            for cb in range(DG // P):
                nc.tensor.matmul(
                    ps[:], oT_sb[:, cb, bass.ts(m, P)], wo_sb[:, cb, bass.ts(n, NQ)],
                    start=(cb == 0), stop=(cb == DG // P - 1))
            ysb = ys.tile([P, NQ], DT, tag="ys", name="ysb")
            if n == 0 or act:
                nc.scalar.activation(ysb[:], ps[:],
                                     mybir.ActivationFunctionType.Copy)
            else:
                nc.vector.tensor_copy(ysb[:], ps[:])
            nc.sync.dma_start(y[bass.ts(m, P), bass.ts(n, NQ)], ysb[:])

        # ---- attention core ----
        # PE emission runs the PV of chunk c-1 after the scores of chunk c so
        # PE has a chunk of slack over the ACT exp chain.
        def emit_att(hi, j, fillers):
            nch = 4 * j + 4
            po = pos.tile([P, 2, NQ], mybir.dt.float32, tag="po", name="po")
            pend = []  # (c, qo, pT) awaiting their PV emission
            nf = len(fillers)
            if hi == 3 and j >= 1:
                # outproj fillers: delay past the first chunks so the DVE
                # recip/norm of the previous j-tile has completed
                popat = set(range(3, min(nch, 3 + nf)))
            else:
                # midpoint spacing (covers the segment tail before boundaries)
                popat = {(2 * i + 1) * nch // (2 * nf) for i in range(nf)}

            def emit_pv():
                c, qo, pT, start, stop = pend.pop(0)
                for s in range(2):
                    nc.tensor.matmul(
                        po[:, s, qo:NQ], v_sb[:, c, 2 * hi + s, :],
                        pT[:, s, qo:NQ],
                        start=start, stop=stop)

            # diagonal chunks first: their post-exp DVE mask latency hides
            # behind the remaining plain chunks of the segment
            order = list(range(4 * j, nch)) + list(range(0, 4 * j))
            for idx, c in enumerate(order):
                qo = max(0, P * c - NQ * j)
                diag = c >= 4 * j
                ps = pss.tile([P, 2, NQ], mybir.dt.float32, tag="ps", name="ps")
                for s in range(2):
                    hb = s * HD
                    nc.tensor.matmul(
                        ps[:, s, qo:NQ],
                        kT_sb[hb:hb + HD, hi, bass.ts(c, P)],
                        qT_sb[hb:hb + HD, hi, NQ * j + qo:NQ * (j + 1)],
                        start=True, stop=True)
                pT = pt.tile([P, 2, NQ], DT, tag="pT", name="pT")
                nc.scalar.activation(
                    pT[:, :, qo:NQ], ps[:, :, qo:NQ],
                    mybir.ActivationFunctionType.Exp, scale=float(HD) ** -0.5)
                if diag:
                    nc.vector.tensor_tensor(
                        pT[:, :, qo:qo + P], pT[:, :, qo:qo + P], tri_sb[:],
                        mybir.AluOpType.mult)
                pend.append((c, qo, pT, idx == 0, idx == nch - 1))
                if fillers and idx in popat:
                    fillers.pop(0)()
                if idx > 0:
                    emit_pv()
            emit_pv()
            # leftover fillers (over-provisioned slot)
            while fillers:
                fillers.pop(0)()
            # eviction: reciprocal of denominator rows, normalize into oT
            rcp = rc.tile([P, 2, NQ], mybir.dt.float32, tag="rc", name="rcp")
            nc.vector.reciprocal(rcp[HD:P, :, :], po[HD:P, :, :])
            for s in range(2):
                hb = s * HD
                nc.vector.tensor_tensor(
                    oT_sb[hb:hb + HD, hi, bass.ts(j, NQ)],
                    po[0:HD, s, :], rcp[HD:P, s, :], mybir.AluOpType.mult)

        # ---- (3,3): per-m PV stops so the tail outproj overlaps the last
        # chunks; the final y tiles are evicted in halves on DVE ----
        # hi=3 segments: PV splits into per-m (128-col) pieces with individual
        # stop chunks, so each oT column block is evicted and its outproj unit
        # emitted as soon as its last k-chunk lands -- no cross-segment
        # deferral, no stall on whole-tile recip/norm.
        def emit_att3(j, fillers):
            hi, nch = 3, 4 * j + 4
            po = pos.tile([P, 2, NQ], mybir.dt.float32, tag="po", name="po3")
            rcp = rc.tile([P, 2, NQ], mybir.dt.float32, tag="rc", name="rcpt")
            nf = len(fillers)
            # most fillers early, two late pops cover the per-m eviction
            # region, one leftover covers the final eviction
            popat = {3, 5, 7, 9, 11, 13, 14}
            pend = []
            pend_ops = []

            def evict_m(mi):
                mo = slice(mi * P, (mi + 1) * P)
                nc.vector.reciprocal(rcp[HD:P, :, mo], po[HD:P, :, mo])
                for s in range(2):
                    hb = s * HD
                    nc.vector.tensor_tensor(
                        oT_sb[hb:hb + HD, 3, bass.ts(4 * j + mi, P)],
                        po[0:HD, s, mo], rcp[HD:P, s, mo], mybir.AluOpType.mult)

            def emit_pv():
                # single accumulation group per PSUM bank (zero-region rule)
                c, qo, pT, start, stop = pend.pop(0)
                for s in range(2):
                    nc.tensor.matmul(
                        po[:, s, qo:NQ], v_sb[:, c, 2 * hi + s, :],
                        pT[:, s, qo:NQ],
                        start=start, stop=stop)

            order = list(range(4 * j, nch)) + list(range(0, 4 * j))
            for idx, c in enumerate(order):
                qo = max(0, P * c - NQ * j)
                diag = c >= 4 * j
                ps = pss.tile([P, 2, NQ], mybir.dt.float32, tag="ps", name="ps")
                for s in range(2):
                    hb = s * HD
                    nc.tensor.matmul(
                        ps[:, s, qo:NQ],
                        kT_sb[hb:hb + HD, hi, bass.ts(c, P)],
                        qT_sb[hb:hb + HD, hi, NQ * j + qo:NQ * (j + 1)],
                        start=True, stop=True)
                pT = pt.tile([P, 2, NQ], DT, tag="pT", name="pT")
                nc.scalar.activation(
                    pT[:, :, qo:NQ], ps[:, :, qo:NQ],
                    mybir.ActivationFunctionType.Exp, scale=float(HD) ** -0.5)
                if diag:
                    nc.vector.tensor_tensor(
                        pT[:, :, qo:qo + P], pT[:, :, qo:qo + P], tri_sb[:],
                        mybir.AluOpType.mult)
                pend.append((c, qo, pT, idx == 0, idx == nch - 1))
                if fillers and idx in popat:
                    fillers.pop(0)()
                if idx > 0:
                    emit_pv()
            emit_pv()
            while fillers:   # leftover fillers cover the eviction latency
                fillers.pop(0)()
            # per-m evictions all emitted first (DVE streams them while PE
            # runs the units); unit copies on ACT (idle at the tail)
            for mi in range(4):
                evict_m(mi)
            for mi in range(4):
                emit_outproj_unit(4 * j + mi, 0, act=True)
                emit_outproj_unit(4 * j + mi, 1, act=True)

        # ---- main interleaved schedule ----
        def F_v(m):
            return lambda: emit_v(m)

        def F_qk(i, jj, qk):
            return lambda: emit_qk_tile(i, jj, qk)

        def F_op(m, n):
            return lambda: emit_outproj_unit(m, n)

        # filler plan per (hi, j):
        FILL = {
            # hi=0: QK jj1-3 of d-block 0, remaining V chunks, QK d-block 1
            (0, 0): [F_qk(0, 1, 0), F_qk(0, 1, 1), F_v(4), F_v(5)],
            (0, 1): [F_qk(0, 2, 0), F_qk(0, 2, 1), F_v(6), F_v(7),
                     F_v(8), F_qk(0, 3, 0), F_qk(0, 3, 1), F_v(9)],
            (0, 2): [F_v(m) for m in range(10, 16)]
                    + [F_qk(1, 0, 0), F_qk(1, 0, 1)],
            (0, 3): [F_qk(1, jj, qk) for jj in range(1, 4) for qk in range(2)],
            (1, 0): [F_qk(2, 0, 0), F_qk(2, 0, 1)],
            (1, 1): [F_qk(2, 1, 0), F_qk(2, 1, 1)],
            (1, 2): [F_qk(2, 2, 0), F_qk(2, 2, 1)],
            (1, 3): [F_qk(2, 3, 0), F_qk(2, 3, 1)],
            (2, 0): [F_qk(3, 0, 0), F_qk(3, 0, 1)],
            (2, 1): [F_qk(3, 1, 0)],
            (2, 2): [F_qk(3, 2, 0), F_qk(3, 2, 1)],
            (2, 3): [F_qk(3, 3, 0), F_qk(3, 3, 1)],
            (3, 0): [F_qk(3, 1, 1)],
            # outproj(j-1) rides inside att(3, j); outproj(3) inline in att3
            (3, 1): [F_op(m, n) for m in range(0, 4) for n in range(2)],
            (3, 2): [F_op(m, n) for m in range(4, 8) for n in range(2)],
            (3, 3): [F_op(m, n) for m in range(8, 12) for n in range(2)],
        }

        for hi in range(NHI):
            for j in range(NJ):
                if hi == 3 and j == 3:
                    emit_att3(j, list(FILL[(hi, j)]))
                else:
                    emit_att(hi, j, list(FILL[(hi, j)]))

    split_waits(nc)
    return nc


def kernel(x, Wq, Wk, Wv, Wo, bo):
    x, Wq, Wk, Wv, Wo, bo = (np.asarray(a, np.float32) for a in (x, Wq, Wk, Wv, Wo, bo))
    if "nc" not in _CACHE:
        _CACHE["nc"] = build()
    nc = _CACHE["nc"]

    # scores^T layout: partition p = key index, free i = query index.
    # keep q >= k: multiply exp'd scores by ones where i >= p (both s-planes)
    tri = np.repeat(
        (np.arange(P)[:, None] <= np.arange(P)[None, :])[:, None, :], 2,
        axis=1).astype(NPDT)
    in_maps = []
    for core in range(8):
        b, g = core // 2, core % 2
        sl = slice(g * DG, (g + 1) * DG)
        in_maps.append({
            "xT": np.ascontiguousarray(x[b].T).astype(NPDT),
            "wqkv": np.ascontiguousarray(
                np.concatenate([Wq[:, sl], Wk[:, sl], Wv[:, sl]], axis=1)).astype(NPDT),
            "wo": np.ascontiguousarray(Wo[sl, :]).astype(NPDT),
            "tri": np.ascontiguousarray(tri),
        })
    res = run_bass_kernel_spmd(nc, in_maps, list(range(8)))
    out = np.empty((B, S, D), np.float32)
    for b in range(B):
        out[b] = (res.results[2 * b]["y"].astype(np.float32)
                  + res.results[2 * b + 1]["y"].astype(np.float32) + bo)
    return out



# revision 6
# speedup vs baseline: 1.0021x; 1.0021x over previous
"""Causal self-attention Trainium2 kernel (B=4, S=2048, D=1024, H=16).

Sharding: 8 cores = 4 batches x 2 head-groups (8 heads each).
Megatron-style: column-parallel QKV, row-parallel output projection;
the 2-way partial-sum reduce + bias happens on host at gather time.

Schedule (per core), engineered for the TRN2 timeline model (PE cost =
out-free-size x cycles-per-row, fp8 DoubleRow = 0.5 cy/row over 2 K-tiles,
ACT cost = free-size, per-instruction access-latency overheads):

  - QKV projections in 3-term hi/lo fp8 (e4m3) DoubleRow:
      x ~ xhi+xlo (4x pre-scale), W ~ Whi+Wlo (64x pre-scale);
      x@W ~ [hh: (Whi_c,Whi_c+1)x(xhi_c,xhi_c+1) chunk-paired]
          + [hl+lh: (Whi_c,Wlo_c)x(xlo_c,xhi_c) same-chunk paired].
      25% fewer PE cycles than fp16 at ~1.8e-3 rel error.
  - Scores fp16 as in baseline: per chunk [128 keys, 2 s-planes, NQ]; exp on
    ACT; diagonal chunks first with post-exp DVE tri-mask.
  - PV TRANSPOSED: queries on PSUM partitions, out free = 65 (64 v-dims +
    ones column for the softmax denominator): po_[q, m, s, 0:65] accumulates
    over chunks (multi accumulation groups per PSUM zero-region; the first
    matmul per bank starts/zeroes it). ~half the fp16 PV cost.
  - Eviction per (hi,j): DVE reciprocal of denom column, per-partition-scalar
    normalize to fp16, PE-transpose [128q x (2s*64)] back to oT layout, DVE
    copy to SBUF. Output projection unchanged (fp16, row-parallel).
  - Projections/outproj emitted as fillers between attention chunks.
  - y evicted as fp16 partials at 256x scale; host sums core pairs, divides
    by 256, adds bias in fp32.
"""
import numpy as np
from contextlib import ExitStack

import concourse.bass as bass
import concourse.tile as tile
import concourse.mybir as mybir
from concourse.bass_utils import run_bass_kernel_spmd

B, S, D, H = 4, 2048, 1024, 16
HD = 64          # head dim
HPC = 8          # heads per core
DG = HPC * HD    # 512 dims per head-group
P = 128
NQ = 512         # q-tile width
NCH = S // P     # 16 k-chunks
NJ = S // NQ     # 4 q-tiles
NHI = HPC // 2   # 4 head-pairs per core
DCH = D // P     # 8 contraction chunks
DT = mybir.dt.float16
F8 = mybir.dt.float8e4
NPDT = np.float16
SX = 4.0         # x pre-scale for fp8 hi/lo
SW = 64.0        # W pre-scale for fp8 hi/lo
ESCALE = float(HD) ** -0.5 / (SX * SX * SW * SW)  # exp scale: undo qk scaling

_CACHE = {}


def split_waits(nc, maxw=1):
    """walrus here accepts at most 1 sync-wait per instruction; split extras onto NOPs."""
    for fn in nc.m.functions:
        for bb in fn.blocks:
            insts = list(bb.instructions)
            new_list = []
            changed = False
            for inst in insts:
                si = inst.sync_info
                waits = list(si.on_wait) if si and si.on_wait else []
                if len(waits) > maxw:
                    changed = True
                    head, keep = waits[:-maxw], waits[-maxw:]
                    for i in range(0, len(head), maxw):
                        nop = mybir.InstNoOp(
                            name=f"{inst.name}_wsplit{i}",
                            sync_info=mybir.SyncInfo(on_wait=head[i:i + maxw], on_update=[]),
                            bass_nofuse=True, engine=inst.engine)
                        nc.register_instruction(nop)
                        new_list.append(nop)
                    inst.sync_info = mybir.SyncInfo(
                        on_wait=keep,
                        on_update=list(si.on_update) if si.on_update else [])
                new_list.append(inst)
            if changed:
                bb.instructions = new_list


def build():
    nc = bass.Bass(trn_type="TRN2", target_bir_lowering=False, debug=False)
    # xT8: [DCH, P, 2(lo,hi), S] fp8 of 4*x^T; wqkv8: [DCH, P, 2(hi,lo), 3DG]
    xT = nc.dram_tensor("xT", [DCH, P, 2, S], F8, kind="ExternalInput").ap()
    wqkv = nc.dram_tensor("wqkv", [DCH, P, 2, 3 * DG], F8, kind="ExternalInput").ap()
    wo = nc.dram_tensor("wo", [DG, D], DT, kind="ExternalInput").ap()
    tri = nc.dram_tensor("tri", [P, 2, P], DT, kind="ExternalInput").ap()
    ident = nc.dram_tensor("ident", [P, P], DT, kind="ExternalInput").ap()
    y = nc.dram_tensor("y", [S, D], DT, kind="ExternalOutput").ap()

    DR = mybir.MatmulPerfMode.DoubleRow

    with tile.TileContext(nc) as tc, ExitStack() as ctx:
        sb = ctx.enter_context(tc.tile_pool(name="sb", bufs=1))
        # PSUM: pp 2x1 bank + pss 2x2 banks + pos 1x2 banks = 8 banks
        pp = ctx.enter_context(tc.tile_pool(name="pp", bufs=2, space="PSUM"))
        pss = ctx.enter_context(tc.tile_pool(name="pss", bufs=2, space="PSUM"))
        pos = ctx.enter_context(tc.tile_pool(name="pos", bufs=1, space="PSUM"))
        pt = ctx.enter_context(tc.tile_pool(name="pt", bufs=6))
        rc = ctx.enter_context(tc.tile_pool(name="rc", bufs=2))
        og = ctx.enter_context(tc.tile_pool(name="og", bufs=2))
        ys = ctx.enter_context(tc.tile_pool(name="ys", bufs=4))

        # ---- resident SBUF tiles ----
        xT_sb = sb.tile([P, DCH, 2, S], F8)          # (lo, hi)
        wqkv_sb = sb.tile([P, DCH, 2, 3 * DG], F8)   # (hi, lo)
        wo_sb = sb.tile([P, DG // P, D], DT)
        tri_sb = sb.tile([P, 2, P], DT)
        id_sb = sb.tile([P, P], DT)
        qT_sb = sb.tile([P, NHI, S], DT)   # [2-head dims, pair, s]
        kT_sb = sb.tile([P, NHI, S], DT)
        v_sb = sb.tile([P, NCH, HPC, 65], DT)  # [k part, chunk, head, V|one]
        oT_sb = sb.tile([P, NHI, S], DT)

        # ---- input DMA stream (ordered for earliest compute start) ----
        for c in range(DCH):
            nc.sync.dma_start(wqkv_sb[:, c], wqkv[c])
            nc.sync.dma_start(xT_sb[:, c, :, 0:NQ], xT[c][:, :, 0:NQ])
        HS = S // 2
        for c in range(DCH):
            nc.sync.dma_start(xT_sb[:, c, :, NQ:HS], xT[c][:, :, NQ:HS])
        nc.sync.dma_start(tri_sb[:], tri[:])
        nc.sync.dma_start(id_sb[:], ident[:])
        for c in range(DCH):
            nc.sync.dma_start(xT_sb[:, c, :, HS:S], xT[c][:, :, HS:S])
        nc.sync.dma_start(wo_sb[:], wo.rearrange("(c p) o -> p c o", p=P))

        # warm-up: tiny matmuls anchor the PE busy-ramp origin early
        warm = sb.tile([P, HD], DT)
        nc.gpsimd.memset(warm[:], 0.0)
        wps = pp.tile([P, NQ], mybir.dt.float32, tag="pp", name="wps")
        for _ in range(12):
            nc.tensor.matmul(wps[0:HD, 0:HD], warm[:, 0:HD], warm[:],
                             start=True, stop=True)

        nc.gpsimd.memset(v_sb[:, :, :, 64:65], 1.0)

        # ---- 3-term hi/lo fp8 projection into a [128, NQ] psum tile ----
        # qk orientation: stationary = W col block (out partitions = w cols),
        # moving = x token cols [xbase : xbase+512].
        def emit_proj_pair(ps, wcol, xbase, cp, first):
            """hh term for chunk pair (2cp, 2cp+1), both 256-col halves."""
            for no in range(2):
                nc.tensor.matmul(
                    ps[:, no * 256: no * 256 + 256],
                    wqkv_sb[:, 2 * cp: 2 * cp + 2, 0, wcol: wcol + P],
                    xT_sb[:, 2 * cp: 2 * cp + 2, 1,
                          xbase + no * 256: xbase + no * 256 + 256],
                    perf_mode=DR, start=(first and no == 0), stop=False,
                    skip_group_check=True)

        def emit_proj_cross(ps, wcol, xbase, c, last):
            """hl+lh term for chunk c, both 256-col halves."""
            for no in range(2):
                nc.tensor.matmul(
                    ps[:, no * 256: no * 256 + 256],
                    wqkv_sb[:, c, 0:2, wcol: wcol + P],
                    xT_sb[:, c, 0:2,
                          xbase + no * 256: xbase + no * 256 + 256],
                    perf_mode=DR, start=False, stop=(last and no == 1),
                    skip_group_check=True)

        # ---- startup: QK d-block 0 jj0 + V chunks 0..3, chunk-paired,
        # chasing the chunked DMA stream ----
        t0 = pp.tile([P, NQ], mybir.dt.float32, tag="pp", name="su_a")
        t1 = pp.tile([P, NQ], mybir.dt.float32, tag="pp", name="su_b")
        t2 = pss.tile([P, 2, NQ], mybir.dt.float32, tag="ps", name="su_c")
        t3 = pss.tile([P, 2, NQ], mybir.dt.float32, tag="ps", name="su_d")
        qacc = [t0, t1]
        vacc = [t2[:, 0], t2[:, 1], t3[:, 0], t3[:, 1]]
        # V lags QK by one chunk-pair; c-outer over psum banks chasing DMA.
        for cp in range(DCH // 2 + 1):
            if cp < DCH // 2:
                for t, wcol in ((0, 0), (1, DG)):   # q then k col blocks
                    emit_proj_pair(qacc[t], wcol, 0, cp, cp == 0)
                    emit_proj_cross(qacc[t], wcol, 0, 2 * cp, False)
                    emit_proj_cross(qacc[t], wcol, 0, 2 * cp + 1,
                                    cp == DCH // 2 - 1)
            if cp > 0:
                c0 = 2 * (cp - 1)
                for m in range(4):
                    # v rows block m: stationary = x columns (tokens)
                    for no in range(2):
                        nc.tensor.matmul(
                            vacc[m][:, no * 256: no * 256 + 256],
                            xT_sb[:, c0:c0 + 2, 1, m * P:(m + 1) * P],
                            wqkv_sb[:, c0:c0 + 2, 0,
                                    2 * DG + no * 256: 2 * DG + no * 256 + 256],
                            perf_mode=DR, start=(cp == 1 and no == 0),
                            stop=False, skip_group_check=True)
                    for cc in (c0, c0 + 1):
                        for no in range(2):
                            nc.tensor.matmul(
                                vacc[m][:, no * 256: no * 256 + 256],
                                xT_sb[:, cc, 0:2, m * P:(m + 1) * P],
                                wqkv_sb[:, cc, 0:2,
                                        2 * DG + no * 256: 2 * DG + no * 256 + 256],
                                perf_mode=DR, start=False,
                                stop=(cp == DCH // 2 and cc == c0 + 1 and no == 1),
                                skip_group_check=True)
        for t, dst in enumerate((qT_sb, kT_sb)):
            nc.vector.tensor_copy(dst[:, 0, bass.ts(0, NQ)], qacc[t][:])
        for m in range(4):
            nc.vector.tensor_copy(
                v_sb[:, m, :, 0:HD],
                vacc[m][:].rearrange("p (h d) -> p h d", d=HD))

        # ---- filler emitters ----
        def emit_qk_tile(i, jj, qk):
            """One [128,512] QK projection tile: d-block i, s-range jj."""
            wcol = qk * DG + i * P
            dst = qT_sb if qk == 0 else kT_sb
            ps = pp.tile([P, NQ], mybir.dt.float32, tag="pp", name="qk")
            for cp in range(DCH // 2):
                emit_proj_pair(ps, wcol, jj * NQ, cp, cp == 0)
            for c in range(DCH):
                emit_proj_cross(ps, wcol, jj * NQ, c, c == DCH - 1)
            nc.vector.tensor_copy(dst[:, i, bass.ts(jj, NQ)], ps[:])

        def emit_v(m):
            """V projection for token block m: [128 tokens, 512 v-dims]."""
            ps = pp.tile([P, DG], mybir.dt.float32, tag="pp", name="vp")
            for cp in range(DCH // 2):
                for no in range(2):
                    nc.tensor.matmul(
                        ps[:, no * 256: no * 256 + 256],
                        xT_sb[:, 2 * cp:2 * cp + 2, 1, m * P:(m + 1) * P],
                        wqkv_sb[:, 2 * cp:2 * cp + 2, 0,
                                2 * DG + no * 256: 2 * DG + no * 256 + 256],
                        perf_mode=DR, start=(cp == 0 and no == 0), stop=False,
                        skip_group_check=True)
            for c in range(DCH):
                for no in range(2):
                    nc.tensor.matmul(
                        ps[:, no * 256: no * 256 + 256],
                        xT_sb[:, c, 0:2, m * P:(m + 1) * P],
                        wqkv_sb[:, c, 0:2,
                                2 * DG + no * 256: 2 * DG + no * 256 + 256],
                        perf_mode=DR, start=False,
                        stop=(c == DCH - 1 and no == 1),
                        skip_group_check=True)
            nc.vector.tensor_copy(
                v_sb[:, m, :, 0:HD], ps[:].rearrange("p (h d) -> p h d", d=HD))

        def emit_outproj_unit(m, n, act=False):
            ps = pp.tile([P, NQ], mybir.dt.float32, tag="pp", name="yp")
            for cb in range(DG // P):
                nc.tensor.matmul(
                    ps[:], oT_sb[:, cb, bass.ts(m, P)], wo_sb[:, cb, bass.ts(n, NQ)],
                    start=(cb == 0), stop=(cb == DG // P - 1))
            ysb = ys.tile([P, NQ], DT, tag="ys", name="ysb")
            if act:
                nc.scalar.activation(ysb[:], ps[:],
                                     mybir.ActivationFunctionType.Copy)
            else:
                nc.vector.tensor_copy(ysb[:], ps[:])
            nc.sync.dma_start(y[bass.ts(m, P), bass.ts(n, NQ)], ysb[:])

        # ---- attention core (PV transposed) ----
        def emit_att(hi, j, fillers):
            nch = 4 * j + 4
            # po_: [128 q, m, s, 128-padded(65 used)] = 2 banks
            po = pos.tile([P, 4, 2, P], mybir.dt.float32, tag="po", name="po")
            bank_started = [False, False]
            # last chunk idx (in emission order) writing each m group
            order = list(range(4 * j, nch)) + list(range(0, 4 * j))
            last_idx = {}
            for idx, c in enumerate(order):
                for m in range(max(0, c - 4 * j), 4):
                    last_idx[m] = idx
            pend = []
            nf = len(fillers)
            popat = {(2 * i + 1) * nch // (2 * nf) for i in range(nf)} if nf else set()

            def emit_pv():
                idx, c, qo, pT = pend.pop(0)
                m0 = max(0, c - 4 * j)
                for s in range(2):
                    for m in range(m0, 4):
                        bank = m // 2
                        st = not bank_started[bank]
                        bank_started[bank] = True
                        nc.tensor.matmul(
                            po[:, m, s, 0:65],
                            pT[:, s, m * P:(m + 1) * P],
                            v_sb[:, c, 2 * hi + s, :],
                            start=st, stop=(last_idx[m] == idx),
                            skip_group_check=True)

            for idx, c in enumerate(order):
                qo = max(0, P * c - NQ * j)
                diag = c >= 4 * j
                ps = pss.tile([P, 2, NQ], mybir.dt.float32, tag="ps", name="ps")
                for s in range(2):
                    hb = s * HD
                    nc.tensor.matmul(
                        ps[:, s, qo:NQ],
                        kT_sb[hb:hb + HD, hi, bass.ts(c, P)],
                        qT_sb[hb:hb + HD, hi, NQ * j + qo:NQ * (j + 1)],
                        start=True, stop=True)
                pT = pt.tile([P, 2, NQ], DT, tag="pT", name="pT")
                nc.scalar.activation(
                    pT[:, :, qo:NQ], ps[:, :, qo:NQ],
                    mybir.ActivationFunctionType.Exp, scale=ESCALE)
                if diag:
                    nc.vector.tensor_tensor(
                        pT[:, :, qo:qo + P], pT[:, :, qo:qo + P], tri_sb[:],
                        mybir.AluOpType.mult)
                pend.append((idx, c, qo, pT))
                if fillers and idx in popat:
                    fillers.pop(0)()
                if idx > 0:
                    emit_pv()
            emit_pv()
            while fillers:
                fillers.pop(0)()
            # eviction: recip of denom col, normalize (fp16), PE-transpose
            rcp = rc.tile([P, 4, 2, 1], mybir.dt.float32, tag="rc", name="rcp")
            nc.vector.reciprocal(rcp[:], po[:, :, :, 64:65])
            o16 = og.tile([P, 4, 2, HD], DT, tag="og", name="o16")
            for m in range(4):
                for s in range(2):
                    nc.vector.tensor_scalar(
                        o16[:, m, s], po[:, m, s, 0:HD], rcp[:, m, s], None,
                        mybir.AluOpType.mult)
            for m in range(4):
                tr = pp.tile([P, P], DT, tag="pp", name="tr")
                nc.tensor.matmul(tr[:], o16[:, m].rearrange("p s d -> p (s d)"),
                                 id_sb[:], is_transpose=True, start=True, stop=True)
                nc.vector.tensor_copy(oT_sb[:, hi, NQ * j + m * P: NQ * j + (m + 1) * P],
                                      tr[:])

        # ---- main interleaved schedule ----
        def F_v(m):
            return lambda: emit_v(m)

        def F_qk(i, jj, qk):
            return lambda: emit_qk_tile(i, jj, qk)

        def F_op(m, n):
            return lambda: emit_outproj_unit(m, n)

        FILL = {
            (0, 0): [F_qk(0, 1, 0), F_qk(0, 1, 1), F_v(4), F_v(5)],
            (0, 1): [F_qk(0, 2, 0), F_qk(0, 2, 1), F_v(6), F_v(7),
                     F_v(8), F_qk(0, 3, 0), F_qk(0, 3, 1), F_v(9)],
            (0, 2): [F_v(m) for m in range(10, 16)]
                    + [F_qk(1, 0, 0), F_qk(1, 0, 1)],
            (0, 3): [F_qk(1, jj, qk) for jj in range(1, 4) for qk in range(2)],
            (1, 0): [F_qk(2, 0, 0), F_qk(2, 0, 1)],
            (1, 1): [F_qk(2, 1, 0), F_qk(2, 1, 1)],
            (1, 2): [F_qk(2, 2, 0), F_qk(2, 2, 1)],
            (1, 3): [F_qk(2, 3, 0), F_qk(2, 3, 1)],
            (2, 0): [F_qk(3, 0, 0), F_qk(3, 0, 1)],
            (2, 1): [F_qk(3, 1, 0)],
            (2, 2): [F_qk(3, 2, 0), F_qk(3, 2, 1)],
            (2, 3): [F_qk(3, 3, 0), F_qk(3, 3, 1)],
            (3, 0): [F_qk(3, 1, 1)],
            (3, 1): [F_op(m, n) for m in range(0, 4) for n in range(2)],
            (3, 2): [F_op(m, n) for m in range(4, 8) for n in range(2)],
            (3, 3): [F_op(m, n) for m in range(8, 12) for n in range(2)],
        }

        for hi in range(NHI):
            for j in range(NJ):
                emit_att(hi, j, list(FILL[(hi, j)]))
        # tail: last j-tile's output projection
        for m in range(12, 16):
            emit_outproj_unit(m, 0, act=True)
            emit_outproj_unit(m, 1, act=True)

    split_waits(nc)
    return nc


def _hilo(a, order):
    """fp8 hi/lo split along a new axis; order 'hl' or 'lh'."""
    import ml_dtypes
    hi = a.astype(ml_dtypes.float8_e4m3)
    lo = (a - hi.astype(np.float32)).astype(ml_dtypes.float8_e4m3)
    pair = (hi, lo) if order == "hl" else (lo, hi)
    return np.stack(pair, axis=-2)


def kernel(x, Wq, Wk, Wv, Wo, bo):
    x, Wq, Wk, Wv, Wo, bo = (np.asarray(a, np.float32) for a in (x, Wq, Wk, Wv, Wo, bo))
    if "nc" not in _CACHE:
        _CACHE["nc"] = build()
    nc = _CACHE["nc"]

    tri = np.repeat(
        (np.arange(P)[:, None] <= np.arange(P)[None, :])[:, None, :], 2,
        axis=1).astype(NPDT)
    ident = np.eye(P, dtype=NPDT)
    in_maps = []
    for core in range(8):
        b, g = core // 2, core % 2
        sl = slice(g * DG, (g + 1) * DG)
        xTc = np.ascontiguousarray((SX * x[b].T).reshape(DCH, P, S))
        wq = np.concatenate([Wq[:, sl], Wk[:, sl], Wv[:, sl]], axis=1)
        wqc = np.ascontiguousarray((SW * wq).reshape(DCH, P, 3 * DG))
        in_maps.append({
            "xT": np.ascontiguousarray(_hilo(xTc, "lh")),
            "wqkv": np.ascontiguousarray(_hilo(wqc, "hl")),
            "wo": np.ascontiguousarray(Wo[sl, :]).astype(NPDT),
            "tri": np.ascontiguousarray(tri),
            "ident": ident,
        })
    res = run_bass_kernel_spmd(nc, in_maps, list(range(8)))
    out = np.empty((B, S, D), np.float32)
    inv = 1.0 / (SX * SW)
    for b in range(B):
        out[b] = (res.results[2 * b]["y"].astype(np.float32)
                  + res.results[2 * b + 1]["y"].astype(np.float32)) * inv + bo
    return out


# revision 59
# speedup vs baseline: 1.1658x; 1.1634x over previous
"""Causal self-attention Trainium2 kernel (B=4, S=2048, D=1024, H=16).

Sharding: 8 cores = 4 batches x 2 head-groups (8 heads each).
Megatron-style: column-parallel QKV, row-parallel output projection;
the 2-way partial-sum reduce + bias happens on host at gather time.

Schedule (per core), engineered for the TRN2 timeline model (PE cost =
out-free-size x cycles-per-row, fp8 DoubleRow = 0.5 cy/row over 2 K-tiles,
ACT cost = free-size, per-instruction access-latency overheads):

  - QKV projections in 3-term hi/lo fp8 (e4m3) DoubleRow:
      x ~ xhi+xlo (4x pre-scale), W ~ Whi+Wlo (64x pre-scale);
      x@W ~ [hh: (Whi_c,Whi_c+1)x(xhi_c,xhi_c+1) chunk-paired]
          + [hl+lh: (Whi_c,Wlo_c)x(xlo_c,xhi_c) same-chunk paired].
      25% fewer PE cycles than fp16 at ~1.8e-3 rel error.
  - Scores fp16: per chunk [128 keys, 2 s-planes, NQ]; exp on ACT; chunk
    order plains-first so diag tri-masks (on the otherwise-idle GPSIMD/Pool
    engine) never gate the PV chain.
  - PV TRANSPOSED: queries on PSUM partitions, out free = 65 (64 v-dims +
    ones column for the softmax denominator): po_[q, m, s, 0:65] accumulates
    over chunks (multi accumulation groups per PSUM zero-region; the first
    matmul per bank starts/zeroes it; m-descending so bank starts are
    tile-ordered vs the previous segment's readers). ~half the fp16 PV cost.
  - Eviction per (hi,j): DVE reciprocal of the denominator column, ONE
    broadcast tensor_tensor normalize to fp16, PE-transpose [128q x (2s*64)]
    back to oT layout, one DVE copy. Deferred into the NEXT segment's first
    two chunk slots (inject stage 1/2) so it never stalls ACT at segment
    boundaries.
  - Output projection row-parallel in 3-term hi/lo fp8 DoubleRow (oT at
    16x scale via a 16.0 ones-column folded into the denominator; Wo at
    64x), split cb01-pair / cb23-pair: the cb01 partial of each
    [128tok x 512] unit runs as soon as hi<=1's oT columns exist (fills
    the dry region), parks in ysum (SBUF fp16); cb23 + add + eviction
    after hi=3's per-m evictions, the last segment pipelining outproj
    inline per completed m-group. Evictions write oT as fp8 (hi, lo)
    pairs (copy + subtract) straight from the transpose PSUM.
  - All projection/outproj work is sliced into ~300-650ns quanta in a
    deadline+availability queue, drained per chunk by an emission-time
    pacer (PE-ns vs ACT-ns debt, larger lead in the PE-bound hi=3 region)
    so PE fills the per-chunk idle under the ACT exp cadence.
  - Input DMA: wqkv split into wqkva (wq blk0|wk blk0|wv -> wave 0, chunk-
    pair-merged DMAs the startup chases) and wqkvb (rest, streamed later);
    V0/V1 accumulate inside the startup DMA-chase gaps, V2+ are paced
    fillers.
  - y evicted as fp16 partials at 1024x scale; host sums core pairs,
    divides by 1024, adds bias in fp32.

Timeline-sim: 208195 ns/core (baseline 242723). HW rel_err 2.16e-3.
"""
import numpy as np
from contextlib import ExitStack

import concourse.bass as bass
import concourse.tile as tile
import concourse.mybir as mybir
from concourse.bass_utils import run_bass_kernel_spmd

B, S, D, H = 4, 2048, 1024, 16
HD = 64          # head dim
HPC = 8          # heads per core
DG = HPC * HD    # 512 dims per head-group
P = 128
NQ = 512         # q-tile width
NCH = S // P     # 16 k-chunks
NJ = S // NQ     # 4 q-tiles
NHI = HPC // 2   # 4 head-pairs per core
DCH = D // P     # 8 contraction chunks
DT = mybir.dt.float16
F8 = mybir.dt.float8e4
NPDT = np.float16
SX = 4.0         # x pre-scale for fp8 hi/lo
SW = 64.0        # W pre-scale for fp8 hi/lo
ESCALE = float(HD) ** -0.5 / (SX * SX * SW * SW)  # exp scale: undo qk scaling

_CACHE = {}


def split_waits(nc, maxw=1):
    """walrus here accepts at most 1 sync-wait per instruction; split extras onto NOPs."""
    for fn in nc.m.functions:
        for bb in fn.blocks:
            insts = list(bb.instructions)
            new_list = []
            changed = False
            for inst in insts:
                si = inst.sync_info
                waits = list(si.on_wait) if si and si.on_wait else []
                if len(waits) > maxw:
                    changed = True
                    head, keep = waits[:-maxw], waits[-maxw:]
                    for i in range(0, len(head), maxw):
                        nop = mybir.InstNoOp(
                            name=f"{inst.name}_wsplit{i}",
                            sync_info=mybir.SyncInfo(on_wait=head[i:i + maxw], on_update=[]),
                            bass_nofuse=True, engine=inst.engine)
                        nc.register_instruction(nop)
                        new_list.append(nop)
                    inst.sync_info = mybir.SyncInfo(
                        on_wait=keep,
                        on_update=list(si.on_update) if si.on_update else [])
                new_list.append(inst)
            if changed:
                bb.instructions = new_list


def build():
    nc = bass.Bass(trn_type="TRN2", target_bir_lowering=False, debug=False)
    # xT8: [DCH, P, 2(lo,hi), S] fp8 of 4*x^T; wqkv8: [DCH, P, 2(hi,lo), 3DG]
    xT = nc.dram_tensor("xT", [DCH, P, 2, S], F8, kind="ExternalInput").ap()
    wqkva = nc.dram_tensor("wqkva", [DCH, P, 2, 768], F8, kind="ExternalInput").ap()
    wqkvb = nc.dram_tensor("wqkvb", [DCH, P, 2, 768], F8, kind="ExternalInput").ap()
    wo = nc.dram_tensor("wo", [DG // P, P, 2, D], F8, kind="ExternalInput").ap()
    tri = nc.dram_tensor("tri", [P, 2, P], DT, kind="ExternalInput").ap()
    ident = nc.dram_tensor("ident", [P, P], DT, kind="ExternalInput").ap()
    y = nc.dram_tensor("y", [S, D], DT, kind="ExternalOutput").ap()

    DR = mybir.MatmulPerfMode.DoubleRow

    with tile.TileContext(nc) as tc, ExitStack() as ctx:
        sb = ctx.enter_context(tc.tile_pool(name="sb", bufs=1))
        # PSUM: pp 2x1 bank + pss 2x2 banks + pos 1x2 banks = 8 banks
        pp = ctx.enter_context(tc.tile_pool(name="pp", bufs=2, space="PSUM"))
        pss = ctx.enter_context(tc.tile_pool(name="pss", bufs=2, space="PSUM"))
        pos = ctx.enter_context(tc.tile_pool(name="pos", bufs=1, space="PSUM"))
        pt = ctx.enter_context(tc.tile_pool(name="pt", bufs=6))
        rc = ctx.enter_context(tc.tile_pool(name="rc", bufs=2))
        og = ctx.enter_context(tc.tile_pool(name="og", bufs=2))
        ys = ctx.enter_context(tc.tile_pool(name="ys", bufs=8))

        # ---- resident SBUF tiles ----
        xT_sb = sb.tile([P, DCH, 2, S], F8)          # (lo, hi)
        wqkva_sb = sb.tile([P, DCH, 2, 768], F8)   # (hi, lo): wq0|wk0|wv
        wqkvb_sb = sb.tile([P, DCH, 2, 768], F8)   # (hi, lo): wq1-3|wk1-3
        wo_sb = sb.tile([P, DG // P, 2, D], F8)   # (hi, lo), 64x pre-scale
        tri_sb = sb.tile([P, 2, P], DT)
        id_sb = sb.tile([P, P], DT)
        qT_sb = sb.tile([P, NHI, S], DT)   # [2-head dims, pair, s]
        kT_sb = sb.tile([P, NHI, S], DT)
        v_sb = sb.tile([P, NCH, HPC, 65], DT)  # [k part, chunk, head, V|one]
        oT_sb = sb.tile([P, NHI, 2, S], F8)   # (lo, hi), 16x scale
        ysum_sb = sb.tile([P, 16, 2, NQ], DT)  # cb0-2 outproj partials

        # ---- input DMA stream (ordered for earliest compute start) ----
        def xt_wave(c0, c1):
            for c in range(DCH):
                nc.sync.dma_start(xT_sb[:, c, :, c0:c1], xT[c][:, :, c0:c1])

        HS = S // 2
        for cp in range(DCH // 2):
            nc.sync.dma_start(
                wqkva_sb[:, 2 * cp:2 * cp + 2],
                wqkva[2 * cp:2 * cp + 2].rearrange("c p h s -> p c h s"))
            nc.sync.dma_start(xT_sb[:, 2 * cp, :, 0:NQ],
                              xT[2 * cp][:, :, 0:NQ])
            nc.sync.dma_start(xT_sb[:, 2 * cp + 1, :, 0:NQ],
                              xT[2 * cp + 1][:, :, 0:NQ])
        nc.sync.dma_start(tri_sb[:], tri[:])
        nc.sync.dma_start(id_sb[:], ident[:])
        xt_wave(NQ, HS)
        xt_wave(HS, HS + NQ)
        for cp in range(DCH // 2):
            nc.sync.dma_start(
                wqkvb_sb[:, 2 * cp:2 * cp + 2],
                wqkvb[2 * cp:2 * cp + 2].rearrange("c p h s -> p c h s"))
        xt_wave(HS + NQ, S)
        nc.sync.dma_start(wo_sb[:], wo.rearrange("c p h o -> p c h o"))

        # warm-up: tiny matmuls anchor the PE busy-ramp origin early
        warm = sb.tile([P, HD], DT)
        nc.gpsimd.memset(warm[:], 0.0)
        wps = pp.tile([P, NQ], mybir.dt.float32, tag="pp", name="wps")
        for _ in range(12):
            nc.tensor.matmul(wps[0:HD, 0:HD], warm[:, 0:HD], warm[:],
                             start=True, stop=True)

        nc.gpsimd.memset(v_sb[:, :, :, 64:65], 16.0)

        # ---- 3-term hi/lo fp8 projection into a [128, NQ] psum tile ----
        # qk orientation: stationary = W col block (out partitions = w cols),
        # moving = x token cols [xbase : xbase+512].
        def emit_proj_pair(ps, wsb, wcol, xbase, cp, first):
            """hh term for chunk pair (2cp, 2cp+1), both 256-col halves."""
            for no in range(2):
                nc.tensor.matmul(
                    ps[:, no * 256: no * 256 + 256],
                    wsb[:, 2 * cp: 2 * cp + 2, 0, wcol: wcol + P],
                    xT_sb[:, 2 * cp: 2 * cp + 2, 1,
                          xbase + no * 256: xbase + no * 256 + 256],
                    perf_mode=DR, start=(first and no == 0), stop=False,
                    skip_group_check=True)

        def emit_proj_cross(ps, wsb, wcol, xbase, c, last):
            """hl+lh term for chunk c, both 256-col halves."""
            for no in range(2):
                nc.tensor.matmul(
                    ps[:, no * 256: no * 256 + 256],
                    wsb[:, c, 0:2, wcol: wcol + P],
                    xT_sb[:, c, 0:2,
                          xbase + no * 256: xbase + no * 256 + 256],
                    perf_mode=DR, start=False, stop=(last and no == 1),
                    skip_group_check=True)

        # ---- startup: QK d-block 0 jj0 + V m=0,1, chasing the W0 DMA
        # pairs; V2..3 are paced fillers during (0,0) ----
        t0 = pp.tile([P, NQ], mybir.dt.float32, tag="pp", name="su_a")
        t1 = pp.tile([P, NQ], mybir.dt.float32, tag="pp", name="su_b")
        t2 = pss.tile([P, 2, NQ], mybir.dt.float32, tag="ps", name="su_c")
        if NVSU > 2:
            t3 = pss.tile([P, 2, NQ], mybir.dt.float32, tag="ps", name="su_d")
        qacc = [t0, t1]
        vslot = [t2[:, 0], t2[:, 1]] + ([t3[:, 0], t3[:, 1]] if NVSU > 2 else [])
        for cp in range(DCH // 2):
            for t, wcol in ((0, 0), (1, P)):   # wq0 then wk0 col blocks
                emit_proj_pair(qacc[t], wqkva_sb, wcol, 0, cp, cp == 0)
                emit_proj_cross(qacc[t], wqkva_sb, wcol, 0, 2 * cp, False)
                emit_proj_cross(qacc[t], wqkva_sb, wcol, 0, 2 * cp + 1,
                                cp == DCH // 2 - 1)
            for m in range(NVSU):
                for no in range(2):
                    nc.tensor.matmul(
                        vslot[m][:, no * 256: no * 256 + 256],
                        xT_sb[:, 2 * cp:2 * cp + 2, 1, m * P:(m + 1) * P],
                        wqkva_sb[:, 2 * cp:2 * cp + 2, 0,
                                 256 + no * 256: 256 + no * 256 + 256],
                        perf_mode=DR, start=(cp == 0 and no == 0),
                        stop=False, skip_group_check=True)
                for cc in (2 * cp, 2 * cp + 1):
                    for no in range(2):
                        nc.tensor.matmul(
                            vslot[m][:, no * 256: no * 256 + 256],
                            xT_sb[:, cc, 0:2, m * P:(m + 1) * P],
                            wqkva_sb[:, cc, 0:2,
                                     256 + no * 256: 256 + no * 256 + 256],
                            perf_mode=DR, start=False,
                            stop=(cp == DCH // 2 - 1 and cc == 2 * cp + 1
                                  and no == 1),
                            skip_group_check=True)
        for t, dst in enumerate((qT_sb, kT_sb)):
            nc.vector.tensor_copy(dst[:, 0, bass.ts(0, NQ)], qacc[t][:])
        for m in range(NVSU):
            nc.vector.tensor_copy(
                v_sb[:, m, :, 0:HD],
                vslot[m][:].rearrange("p (h d) -> p h d", d=HD))

        # ---- filler emitters (paced quanta queue) ----
        # FQ entries: [deadline_seq, avail_seq, cost_ns, fn]. The attention
        # loop drains: (a) anything whose deadline has arrived, (b) by debt:
        # while emitted PE-ns trails emitted ACT-ns. Quanta are ~300-450ns so
        # they pack into the per-chunk PE idle under the exp cadence.
        FQ = []
        BIG = 10 ** 9
        state = {"act": 0.0, "pe": 0.0}

        def seqbase(hi, j):
            return (hi * 4 + j) * 64

        def push(deadline, avail, cost, fn):
            FQ.append([BIG if deadline is None else deadline,
                       -1 if avail is None else avail, cost, fn])

        def push_front(cost, fn):
            FQ.insert(0, [BIG, -1, cost, fn])

        def drain(now, lead=150):
            while FQ and FQ[0][0] <= now:
                _, _, cost, fn = FQ.pop(0)
                fn()
                state["pe"] += cost
            while (FQ and FQ[0][1] <= now
                   and state["pe"] + FQ[0][2] <= state["act"] + lead):
                _, _, cost, fn = FQ.pop(0)
                fn()
                state["pe"] += cost

        def wq_loc(i):
            return (wqkva_sb, 0) if i == 0 else (wqkvb_sb, (i - 1) * P)

        def wk_loc(i):
            return (wqkva_sb, P) if i == 0 else (wqkvb_sb, 384 + (i - 1) * P)

        def qk_quanta(i, jj, qk, dls, avail=None):
            wsb, wcol = wk_loc(i) if qk else wq_loc(i)
            dst = qT_sb if qk == 0 else kT_sb
            cell = {}

            def q_start():
                cell["ps"] = pp.tile([P, NQ], mybir.dt.float32, tag="pp", name="qk")
                for cp in range(DCH // 2):
                    emit_proj_pair(cell["ps"], wsb, wcol, jj * NQ, cp, cp == 0)

            def q_cross(c0, c1, ev):
                def f():
                    for c in range(c0, c1):
                        emit_proj_cross(cell["ps"], wsb, wcol, jj * NQ, c,
                                        c == DCH - 1)
                    if ev:
                        nc.vector.tensor_copy(dst[:, i, bass.ts(jj, NQ)],
                                              cell["ps"][:])
                return f
            push(dls[0], avail, 8 * 53, q_start)
            push(dls[1], avail, 6 * 53, q_cross(0, 3, False))
            push(dls[2], avail, 6 * 53, q_cross(3, 6, False))
            push(dls[3], avail, 4 * 53, q_cross(6, 8, True))

        def v_quanta(m, deadline):
            cell = {}

            def v_pair(c0, c1, first):
                def f():
                    if first:
                        cell["ps"] = pp.tile([P, DG], mybir.dt.float32,
                                             tag="pp", name="vp")
                    for cp in range(c0, c1):
                        for no in range(2):
                            nc.tensor.matmul(
                                cell["ps"][:, no * 256: no * 256 + 256],
                                xT_sb[:, 2 * cp:2 * cp + 2, 1, m * P:(m + 1) * P],
                                wqkva_sb[:, 2 * cp:2 * cp + 2, 0,
                                         256 + no * 256: 256 + no * 256 + 256],
                                perf_mode=DR, start=(first and cp == c0 and no == 0),
                                stop=False, skip_group_check=True)
                return f

            def v_cross(c0, c1, ev):
                def f():
                    for c in range(c0, c1):
                        for no in range(2):
                            nc.tensor.matmul(
                                cell["ps"][:, no * 256: no * 256 + 256],
                                xT_sb[:, c, 0:2, m * P:(m + 1) * P],
                                wqkva_sb[:, c, 0:2,
                                         256 + no * 256: 256 + no * 256 + 256],
                                perf_mode=DR, start=False,
                                stop=(c == DCH - 1 and no == 1),
                                skip_group_check=True)
                    if ev:
                        nc.vector.tensor_copy(
                            v_sb[:, m, :, 0:HD],
                            cell["ps"][:].rearrange("p (h d) -> p h d", d=HD))
                return f
            push(deadline, None, 8 * 53, v_pair(0, 4, True))
            push(deadline, None, 6 * 53, v_cross(0, 3, False))
            push(deadline, None, 6 * 53, v_cross(3, 6, False))
            push(deadline, None, 4 * 53, v_cross(6, 8, True))

        def emit_outproj_unit(m, n, act=False):
            ps = pp.tile([P, NQ], mybir.dt.float32, tag="pp", name="yp")
            for cb in range(DG // P):
                nc.tensor.matmul(
                    ps[:], oT_sb[:, cb, bass.ts(m, P)], wo_sb[:, cb, bass.ts(n, NQ)],
                    start=(cb == 0), stop=(cb == DG // P - 1))
            ysb = ys.tile([P, NQ], DT, tag="ys", name="ysb")
            nc.scalar.activation(ysb[:, 0:NQ // 2], ps[:, 0:NQ // 2],
                                 mybir.ActivationFunctionType.Copy)
            nc.vector.tensor_copy(ysb[:, NQ // 2:], ps[:, NQ // 2:])
            nc.sync.dma_start(y[bass.ts(m, P), bass.ts(n, NQ)], ysb[:])

        def op_pair(ps, cb0, m, n, first, last):
            # 3-term fp8 for cb pair (cb0, cb0+1): hh + cross(cb0) + cross(cb0+1)
            for no in range(2):
                nc.tensor.matmul(
                    ps[:, no * 256: no * 256 + 256],
                    oT_sb[:, cb0:cb0 + 2, 1, m * P:(m + 1) * P],
                    wo_sb[:, cb0:cb0 + 2, 0,
                          n * NQ + no * 256: n * NQ + no * 256 + 256],
                    perf_mode=DR, start=(first and no == 0), stop=False,
                    skip_group_check=True)
            for cb in (cb0, cb0 + 1):
                for no in range(2):
                    nc.tensor.matmul(
                        ps[:, no * 256: no * 256 + 256],
                        oT_sb[:, cb, 0:2, m * P:(m + 1) * P],
                        wo_sb[:, cb, 0:2,
                              n * NQ + no * 256: n * NQ + no * 256 + 256],
                        perf_mode=DR, start=False,
                        stop=(last and cb == cb0 + 1 and no == 1),
                        skip_group_check=True)

        def opA_quanta(m, n, avail):
            # cb0+cb1 partial of unit (m, n): park in ysum
            def f():
                ps = pp.tile([P, NQ], mybir.dt.float32, tag="pp", name="ypa")
                op_pair(ps, 0, m, n, True, True)
                nc.vector.tensor_copy(ysum_sb[:, m, n], ps[:])
            push(None, avail, 320, f)


        TAILMODE = {"on": False}

        def opB(m, n):
            # cb2+cb3 partial + add the cb01 partial, evict, dma
            ps = pp.tile([P, NQ], mybir.dt.float32, tag="pp", name="ypb")
            op_pair(ps, 2, m, n, True, True)
            ysb = ys.tile([P, NQ], DT, tag="ys", name="ysb")
            nc.vector.tensor_tensor(ysb[:], ps[:], ysum_sb[:, m, n],
                                    mybir.AluOpType.add)
            nc.sync.dma_start(y[bass.ts(m, P), bass.ts(n, NQ)], ysb[:])

        def opB_quanta(m, n, avail, deadline=None):
            push(deadline, avail, 420, lambda: opB(m, n))

        def op_quanta(m, n, avail):
            cell = {}

            def h1():
                cell["ps"] = pp.tile([P, NQ], mybir.dt.float32, tag="pp", name="yp")
                for cb in range(2):
                    nc.tensor.matmul(
                        cell["ps"][:], oT_sb[:, cb, bass.ts(m, P)],
                        wo_sb[:, cb, bass.ts(n, NQ)],
                        start=(cb == 0), stop=False)

            def h2():
                for cb in range(2, 4):
                    nc.tensor.matmul(
                        cell["ps"][:], oT_sb[:, cb, bass.ts(m, P)],
                        wo_sb[:, cb, bass.ts(n, NQ)],
                        start=False, stop=(cb == 3))
                ysb = ys.tile([P, NQ], DT, tag="ys", name="ysb")
                nc.vector.tensor_copy(ysb[:], cell["ps"][:])
                nc.sync.dma_start(y[bass.ts(m, P), bass.ts(n, NQ)], ysb[:])
            push(None, avail, 2 * 213, h1)
            push(None, avail, 2 * 213 + 100, h2)

        OPQ = []  # pending outproj unit-halves (hi=3, inline per-m)

        def op_halves(m, n):
            cell = {}

            def h1():
                cell["ps"] = pp.tile([P, NQ], mybir.dt.float32, tag="pp", name="yp")
                for cb in range(2):
                    nc.tensor.matmul(
                        cell["ps"][:], oT_sb[:, cb, bass.ts(m, P)],
                        wo_sb[:, cb, bass.ts(n, NQ)],
                        start=(cb == 0), stop=False)

            def h2():
                for cb in range(2, 4):
                    nc.tensor.matmul(
                        cell["ps"][:], oT_sb[:, cb, bass.ts(m, P)],
                        wo_sb[:, cb, bass.ts(n, NQ)],
                        start=False, stop=(cb == 3))
                ysb = ys.tile([P, NQ], DT, tag="ys", name="ysb")
                nc.scalar.activation(ysb[:, 0:NQ // 2], cell["ps"][:, 0:NQ // 2],
                                     mybir.ActivationFunctionType.Copy)
                nc.vector.tensor_copy(ysb[:, NQ // 2:], cell["ps"][:, NQ // 2:])
                nc.sync.dma_start(y[bass.ts(m, P), bass.ts(n, NQ)], ysb[:])
            return [h1, h2]

        # ---- attention core (PV transposed, plains-first chunk order) ----
        def emit_att(hi, j, inject=None, last=False):
            nch = 4 * j + 4
            state["act"] = float(PRIME)
            state["pe"] = 0.0
            base = seqbase(hi, j)
            po = pos.tile([P, 4, 2, P], mybir.dt.float32, tag="po", name="po")
            bank_started = [False, False]
            order = list(range(0, 4 * j)) + list(range(4 * j, nch))
            last_idx = {}
            for idx, c in enumerate(order):
                for m in range(max(0, c - 4 * j), 4):
                    last_idx[m] = idx
            pend = []
            rcp = rc.tile([P, 4, 2, 1], mybir.dt.float32, tag="rc", name="rcp")
            o16 = og.tile([P, 4, 2, HD], DT, tag="og", name="o16")

            def evict_m(m):
                # per-m eviction (last segment): recip, normalize, transpose
                nc.vector.reciprocal(rcp[:, m], po[:, m, :, 64:65])
                nc.vector.tensor_tensor(
                    o16[:, m], po[:, m, :, 0:HD],
                    rcp[:, m, :, 0:1].to_broadcast([P, 2, HD]),
                    mybir.AluOpType.mult)
                tr = pp.tile([P, P], DT, tag="pp", name="tr3")
                nc.tensor.matmul(tr[:], o16[:, m].rearrange("p s d -> p (s d)"),
                                 id_sb[:], is_transpose=True, start=True, stop=True)
                sl = slice(NQ * j + m * P, NQ * j + (m + 1) * P)
                nc.vector.tensor_copy(oT_sb[:, hi, 1, sl], tr[:])
                nc.vector.tensor_tensor(oT_sb[:, hi, 0, sl], tr[:],
                                        oT_sb[:, hi, 1, sl],
                                        mybir.AluOpType.subtract)
                state["pe"] += 53

            def emit_pv():
                idx, c, pT = pend.pop(0)
                m0 = max(0, c - 4 * j)
                for s in range(2):
                    for m in range(3, m0 - 1, -1):
                        bank = m // 2
                        st = not bank_started[bank]
                        bank_started[bank] = True
                        nc.tensor.matmul(
                            po[:, m, s, 0:65],
                            pT[:, s, m * P:(m + 1) * P],
                            v_sb[:, c, 2 * hi + s, :],
                            start=st, stop=(last_idx[m] == idx),
                            skip_group_check=True)
                state["pe"] += (4 - m0) * 2 * 27
                if last:
                    for m in range(4):
                        if last_idx[m] == idx:
                            evict_m(m)
                            for n in range(2):
                                mm, nn = 4 * j + m, n
                                if mm < NSPLIT:
                                    OPQ.append(
                                        (lambda a, b: lambda: opB(a, b))(mm, nn))
                                else:
                                    OPQ.extend(op_halves(mm, nn))

            for idx, c in enumerate(order):
                qo = max(0, P * c - NQ * j)
                diag = c >= 4 * j
                w = NQ - qo
                ps = pss.tile([P, 2, NQ], mybir.dt.float32, tag="ps", name="ps")
                for s in range(2):
                    hb = s * HD
                    nc.tensor.matmul(
                        ps[:, s, qo:NQ],
                        kT_sb[hb:hb + HD, hi, bass.ts(c, P)],
                        qT_sb[hb:hb + HD, hi, NQ * j + qo:NQ * (j + 1)],
                        start=True, stop=True)
                state["pe"] += 2 * w * 0.4167
                pT = pt.tile([P, 2, NQ], DT, tag="pT", name="pT")
                nc.scalar.activation(
                    pT[:, :, qo:NQ], ps[:, :, qo:NQ],
                    mybir.ActivationFunctionType.Exp, scale=ESCALE)
                state["act"] += 2 * w * 0.8333 + 185
                if diag:
                    nc.gpsimd.tensor_tensor(
                        pT[:, :, qo:qo + P], pT[:, :, qo:qo + P], tri_sb[:],
                        mybir.AluOpType.mult)
                pend.append((idx, c, pT))
                if idx == 0 and inject is not None:
                    inject[0]()
                if idx == 1 and inject is not None:
                    inject[1]()
                drain(base + idx, lead=2000 if hi == 3 else 150)
                if idx > 0:
                    emit_pv()
                npop = 2 if (last and idx > nch // 2) else 1
                for _ in range(npop):
                    if OPQ:
                        OPQ.pop(0)()
                        state["pe"] += 426
            emit_pv()

            def evict():
                # whole-segment eviction, called from the NEXT segment's first
                # chunk so this chain never delays that segment's first exp.
                # Transposes are spread over the next chunks via the queue so
                # they never pile up in PE's wait queue behind the DVE norm.
                nc.vector.reciprocal(rcp[:], po[:, :, :, 64:65])
                nc.vector.tensor_tensor(
                    o16[:], po[:, :, :, 0:HD],
                    rcp[:, :, :, 0:1].to_broadcast([P, 4, 2, HD]),
                    mybir.AluOpType.mult)
            def evict2():
                tr4 = pp.tile([P, 4, P], DT, tag="pp", name="tr4")
                for m in range(4):
                    nc.tensor.matmul(tr4[:, m],
                                     o16[:, m].rearrange("p s d -> p (s d)"),
                                     id_sb[:], is_transpose=True,
                                     start=True, stop=True)
                nc.vector.tensor_copy(oT_sb[:, hi, 1, bass.ts(j, NQ)], tr4[:])
                nc.vector.tensor_tensor(
                    oT_sb[:, hi, 0, bass.ts(j, NQ)], tr4[:],
                    oT_sb[:, hi, 1, bass.ts(j, NQ)], mybir.AluOpType.subtract)
                state["pe"] += 4 * 53
            return evict, evict2

        # ---- queue construction (deadline order) ----
        # qk tile (i, jj) quanta spread over the tail of the PREVIOUS
        # segment; V_m quanta due just before the PV of diag chunk 4j+m.
        segs = [(hi, j) for hi in range(NHI) for j in range(NJ)]

        def qk_deadlines(i, jj):
            pi = segs.index((i, jj)) - 1
            phi, pj = segs[pi]
            pbase, pnch = seqbase(phi, pj), 4 * pj + 4
            if DLSPREAD:
                return [pbase + (q * pnch) // 8 for q in range(8)]
            return [pbase + min(pnch - 1, max(0, pnch - 8) + q) for q in range(8)]

        def push_qk_pair(i, jj, avail=None):
            dls = qk_deadlines(i, jj)
            qk_quanta(i, jj, 0, dls[0:4], avail)
            qk_quanta(i, jj, 1, dls[4:8], avail)

        for m in range(NVSU, 4):
            v_quanta(m, seqbase(0, 0) + m)
        push_qk_pair(0, 1)
        for m in range(4, 8):
            v_quanta(m, seqbase(0, 1) + m)
        push_qk_pair(0, 2)
        for m in range(8, 12):
            v_quanta(m, seqbase(0, 2) + m)
        push_qk_pair(0, 3)
        for m in range(12, 16):
            v_quanta(m, seqbase(0, 3) + m)
        for i in range(1, 4):
            for jj in range(4):
                push_qk_pair(i, jj)
        for j0 in range(4):
            av = seqbase(2, j0 + 1) if j0 < 3 else seqbase(3, 0)
            for m in range(4 * j0, 4 * j0 + 4):
                if m >= NSPLIT:
                    continue
                for n in range(2):
                    opA_quanta(m, n, av)
        for j0 in range(3):
            for m in range(4 * j0, 4 * j0 + 4):
                for n in range(2):
                    # force each B group into the chunk stream of the segment
                    # after it becomes available, spread over its chunks
                    k = 2 * (m % 4) + n
                    if j0 < 2:
                        dl = seqbase(3, j0 + 2) + (k * (4 * (j0 + 2) + 4)) // 8
                    else:
                        dl = seqbase(3, 3) + 8 + k
                    opB_quanta(m, n, seqbase(3, j0 + 1), dl)
        ev = None
        for hi in range(NHI):
            for j in range(NJ):
                ev = emit_att(hi, j, inject=ev, last=(hi == 3 and j == 3))
        TAILMODE["on"] = True
        # final drain of any leftover fillers and outproj halves; unit halves
        # are interleaved (h1 of unit k+1 before h2 of unit k) so each unit's
        # psum eviction overlaps the next unit's matmuls.
        while FQ:
            FQ.pop(0)[3]()
        if OPPIPE:
            if len(OPQ) % 2:
                OPQ.pop(0)()  # stranded h2 of a half-popped unit
            pairs = [(OPQ[i], OPQ[i + 1]) for i in range(0, len(OPQ), 2)]
            for k, (h1, h2) in enumerate(pairs):
                h1()
                if k > 0:
                    pairs[k - 1][1]()
            if pairs:
                pairs[-1][1]()
        else:
            while OPQ:
                OPQ.pop(0)()

    split_waits(nc)
    return nc


def _hilo(a, order):
    """fp8 hi/lo split along a new axis; order 'hl' or 'lh'."""
    import ml_dtypes
    hi = a.astype(ml_dtypes.float8_e4m3)
    lo = (a - hi.astype(np.float32)).astype(ml_dtypes.float8_e4m3)
    pair = (hi, lo) if order == "hl" else (lo, hi)
    return np.stack(pair, axis=-2)


def kernel(x, Wq, Wk, Wv, Wo, bo):
    x, Wq, Wk, Wv, Wo, bo = (np.asarray(a, np.float32) for a in (x, Wq, Wk, Wv, Wo, bo))
    if "nc" not in _CACHE:
        _CACHE["nc"] = build()
    nc = _CACHE["nc"]

    tri = np.repeat(
        (np.arange(P)[:, None] <= np.arange(P)[None, :])[:, None, :], 2,
        axis=1).astype(NPDT)
    ident = np.eye(P, dtype=NPDT)
    in_maps = []
    for core in range(8):
        b, g = core // 2, core % 2
        sl = slice(g * DG, (g + 1) * DG)
        xTc = np.ascontiguousarray((SX * x[b].T).reshape(DCH, P, S))
        wq_g, wk_g, wv_g = Wq[:, sl], Wk[:, sl], Wv[:, sl]
        # wqkva: [wq block0 | wk block0 | wv]; wqkvb: [wq blocks 1-3 | wk 1-3]
        wa = np.concatenate([wq_g[:, :P], wk_g[:, :P], wv_g], axis=1)
        wb = np.concatenate([wq_g[:, P:], wk_g[:, P:]], axis=1)
        wac = np.ascontiguousarray((SW * wa).reshape(DCH, P, 768))
        wbc = np.ascontiguousarray((SW * wb).reshape(DCH, P, 768))
        in_maps.append({
            "xT": np.ascontiguousarray(_hilo(xTc, "lh")),
            "wqkva": np.ascontiguousarray(_hilo(wac, "hl")),
            "wqkvb": np.ascontiguousarray(_hilo(wbc, "hl")),
            "wo": np.ascontiguousarray(
                _hilo((SW * Wo[sl, :]).reshape(DG // P, P, D), "hl")),
            "tri": np.ascontiguousarray(tri),
            "ident": ident,
        })
    res = run_bass_kernel_spmd(nc, in_maps, list(range(8)))
    out = np.empty((B, S, D), np.float32)
    inv = 1.0 / (16.0 * SW)
    for b in range(B):
        out[b] = (res.results[2 * b]["y"].astype(np.float32)
                  + res.results[2 * b + 1]["y"].astype(np.float32)) * inv + bo
    return out
